# revision 43
# baseline (speedup 1.0000x reference)
"""Trainium2 Bass kernel for masked-softmax attention (sparse_attention).

Computes, for full inputs
    x           [H=4, N=4096, D=256] f32
    adj         [N, N] int32 (0/1)
    att_pattern [H, N, N] f32
the reference
    score = leaky_relu(att_pattern, 0.2)
    score = where(adj > 0, score, -9e15)
    ratio = softmax(score, axis=-1)
    out   = einsum('hnm,hmd->hnd', ratio, x)

Sharding: head-parallel — core c owns head c//2, row half c%2 (2048 rows of one
head). Each core reads only its own head's x slab (2.1MB instead of the full
8.4MB), and adj is never shipped: the host folds mask+leaky into the score
tensor s = where(adj, leaky_relu(att), -17), which also removes the on-chip
mask multiply and leaky passes entirely.

The kernel is ACT-bound (exp is 1 elem/cycle/lane at 1.2GHz; 8.4M elems/core
~ 59us) with the PE matmul stream just behind it (~58us), so everything else
must hide under the exp stream:
  - the first U8TILES row-blocks ship as uint8 codes (half the f16 bytes);
    the ACT free affine (out = exp(scale*u + bias)) decodes them at zero cost,
    so the DMA ramp feeds the exp stream without stalling it. Quantization
    touches U8TILES/16 of the rows -> l2 err ~8e-3 within the 2e-2 budget.
  - the last block's exp is split [16,12,3,1] contraction-chunks wide so only
    one matmul chunk + a 64KB store trail the final exp; block 0's exp starts
    on a 512-col piece as soon as the first 64KB lands.
  - no PE clock prewarm and no exp pairing: dummy-matmul warmup flips the PE
    HAM gate early but trips the chip power governor (whole-chip ~20%
    downclock); paired 8192-wide ACTIVATEs create >3.4us PE idle bubbles
    that re-throttle the PE clock. Both measured net losses.
  - output stores ride the second HWDGE ring (nc.scalar) so they never
    displace input tiles on the main FIFO.

Per-core pipeline, per 128-row block b (16 blocks):
    DMA   s[b]  [128, 4096] u8/f16
    ACT   e = exp(scale * s[b] + bias)    (~3.7us)  <- bottleneck engine
    PE    psum[128, 257] = sum_k e_chunk.T @ x_chunk  (32 matmuls, ~3.6us)
    DVE   out_rows = psum[:, :256] * (1 / psum[:, 256])
    DMA   store out rows (batched)
fp16 data path, fp32 PSUM accumulation, f16 output (cast f32 on host).
"""

import numpy as np

import concourse.bass as bass
import concourse.mybir as mybir
import concourse.tile as tile
from concourse import bacc
from concourse.bass_utils import run_bass_kernel_spmd

H, N, D = 4, 4096, 256
NCORES = 8
R = N // 2               # rows per core = 2048 (one head, half the rows)
RBLKS = R // 128         # 128-row blocks per core = 16
KC = N // 128            # contraction chunks = 32
DP1 = D + 1              # matmul rhs width (ones column appended)
NEG = -17.0              # masked score: exp(-17) ~ 4e-8 -> 0 in f16

U8TILES = 8              # leading row-blocks shipped as u8 codes
PREWARM = 0              # dummy matmuls before the real stream (PE clock warmup)
                         # (any value that keeps the PE busy early trips the
                         # chip power governor -> whole-chip downclock; keep 0)
QLO, QHI = -6.2, 5.3     # u8 code range; code 0 = masked (exp(QLO) ~ 2e-3)
QSC = (QHI - QLO) / 254.0

f32 = mybir.dt.float32
f16 = mybir.dt.float16
u8 = mybir.dt.uint8
AF = mybir.ActivationFunctionType
OP = mybir.AluOpType


def _emit(ctx, tc: tile.TileContext, sQ: bass.AP, sT: bass.AP, xb16: bass.AP,
          out: bass.AP):
    nc = tc.nc

    qp = ctx.enter_context(tc.tile_pool(name="qp", bufs=1))
    sp = ctx.enter_context(tc.tile_pool(name="sp", bufs=6))
    ep = ctx.enter_context(tc.tile_pool(name="ep", bufs=6))
    xp = ctx.enter_context(tc.tile_pool(name="xp", bufs=1))
    bp = ctx.enter_context(tc.tile_pool(name="bp", bufs=1))
    op = ctx.enter_context(tc.tile_pool(name="op", bufs=2))
    rp = ctx.enter_context(tc.tile_pool(name="rp", bufs=2))
    pp = ctx.enter_context(tc.tile_pool(name="pp", bufs=8, space="PSUM"))

    bt = bp.tile([128, 1], f32, tag="bias")
    nc.vector.memset(bt, QLO)
    # Tiny dummy exp: walrus attaches the ~2.7us exp ACT_TABLE_LOAD to the
    # first ACTIVATE — doing one on a ready tile prefetches the table while
    # the input DMA is still ramping, so the first real exp isn't delayed.
    warm_e = bp.tile([128, 1], f32, tag="warme")
    nc.scalar.activation(warm_e, bt, AF.Exp)

    if PREWARM:
        # PE prewarm: dummy matmuls before any input data arrives, to flip
        # the PE HAM clock-gate early. Left disabled: the chip power governor
        # responds with a whole-chip downclock that costs far more.
        wt = bp.tile([128, 128], f16, tag="warm")
        nc.vector.memset(wt, 0.0)
        wpo = pp.tile([128, DP1], f32, tag="po")
        for _ in range(PREWARM):
            nc.tensor.matmul(wpo[:, :128], lhsT=wt, rhs=wt, start=True,
                             stop=True)

    xs = xp.tile([128, KC, DP1], f16, tag="xs")
    xv = xb16.rearrange("p (k d) -> p k d", k=KC)

    # Input FIFO order: first u8 half starts ACT as early as possible; x
    # chunks are wedged between the u8 tiles just ahead of the PE's needs
    # (the PE runs its first ~3.4us at half clock, so x can trickle).
    sq = qp.tile([128, U8TILES, N], u8, tag="sq")
    HN = N // 2
    QN = 512             # first exp piece: small, so ACT starts ASAP
    # All inputs ride ONE HWDGE ring in need-order. (Measured dead ends: a
    # second concurrent ring round-robins at packet granularity and destroys
    # this ordering, +16us of stalls; 64KB-granularity transfers are
    # descriptor-dominated and halve the early rate.)
    nc.sync.dma_start(sq[:, 0, :QN], sQ[0][:, :QN])
    nc.sync.dma_start(sq[:, 0, QN:HN], sQ[0][:, QN:HN])
    nc.sync.dma_start(sq[:, 0, HN:], sQ[0][:, HN:])
    nc.sync.dma_start(xs[:, :6, :], xv[:, :6, :])
    nc.sync.dma_start(sq[:, 1, :], sQ[1])
    nc.sync.dma_start(xs[:, 6:14, :], xv[:, 6:14, :])
    nc.sync.dma_start(sq[:, 2, :], sQ[2])
    nc.sync.dma_start(xs[:, 14:24, :], xv[:, 14:24, :])
    nc.sync.dma_start(sq[:, 3, :], sQ[3])
    nc.sync.dma_start(xs[:, 24:, :], xv[:, 24:, :])
    for i in range(4, U8TILES):
        nc.sync.dma_start(sq[:, i, :], sQ[i])

    ob = {}

    def norm_store(b, po):
        rec = rp.tile([128, 1], f32, tag="rec")
        nc.vector.reciprocal(rec, po[:, D:DP1])
        if b % 4 == 0:
            ob[0] = op.tile([128, 4, D], f16, tag="o", name=f"ob{b}")
        nc.vector.tensor_scalar_mul(ob[0][:, b % 4, :], po[:, :D], rec)
        if b == RBLKS - 2:
            # ship blocks 12-14 early so only 64KB trails the last block
            nc.scalar.dma_start(
                out[12 * 128:15 * 128].rearrange("(rb p) d -> p rb d", p=128),
                ob[0][:, :3, :])
        elif b == RBLKS - 1:
            # final 64KB store split across both HWDGE rings: the two HBM
            # write-receipt latencies overlap instead of chaining
            nc.scalar.dma_start(
                out[15 * 128:].rearrange("(rb p) d -> p rb d", p=128)[:, :, :D // 2],
                ob[0][:, 3:4, :D // 2])
            nc.sync.dma_start(
                out[15 * 128:].rearrange("(rb p) d -> p rb d", p=128)[:, :, D // 2:],
                ob[0][:, 3:4, D // 2:])
        elif b % 4 == 3:
            g = b // 4
            nc.scalar.dma_start(
                out[g * 512:(g + 1) * 512].rearrange("(rb p) d -> p rb d",
                                                     p=128),
                ob[0])

    def mm(po, e_blk, k0, k1):
        for kk in range(k0, k1):
            nc.tensor.matmul(po, lhsT=e_blk[:, kk * 128:(kk + 1) * 128],
                             rhs=xs[:, kk, :], start=(kk == 0),
                             stop=(kk == KC - 1))

    # u8 zone: singles (supply-limited during the DMA ramp); block 0 halved.
    for b in range(U8TILES):
        st = sq[:, b, :]
        e = ep.tile([128, N], f16, tag="e")
        po = pp.tile([128, DP1], f32, tag="po")
        if b == 0:
            nc.scalar.activation(e[:, :QN], st[:, :QN], AF.Exp, scale=QSC,
                                 bias=bt)
            nc.scalar.activation(e[:, QN:], st[:, QN:], AF.Exp, scale=QSC,
                                 bias=bt)
        else:
            nc.scalar.activation(e, st, AF.Exp, scale=QSC, bias=bt)
        mm(po, e, 0, KC)
        norm_store(b, po)

    # f16 zone: single tiles.
    for b in range(U8TILES, RBLKS - 1):
        st = sp.tile([128, N], f16, tag="s")
        nc.sync.dma_start(st, sT[b - U8TILES])
        e = ep.tile([128, N], f16, tag="e")
        nc.scalar.activation(e, st, AF.Exp)
        po = pp.tile([128, DP1], f32, tag="po")
        mm(po, e, 0, KC)
        norm_store(b, po)

    # last block: uneven exp split so only one matmul chunk trails the exp
    b = RBLKS - 1
    st = sp.tile([128, N], f16, tag="s")
    nc.sync.dma_start(st, sT[b - U8TILES])
    e = ep.tile([128, N], f16, tag="e")
    po = pp.tile([128, DP1], f32, tag="po")
    cuts = [0, 16, 28, 31, KC]
    for ci in range(4):
        k0, k1 = cuts[ci], cuts[ci + 1]
        nc.scalar.activation(e[:, k0 * 128:k1 * 128],
                             st[:, k0 * 128:k1 * 128], AF.Exp)
        mm(po, e, k0, k1)
    norm_store(b, po)


def _build():
    from contextlib import ExitStack

    nc = bacc.Bacc(None, target_bir_lowering=False)
    # s*[rb, p, k*128 + r] = s[rb*128 + r, k*128 + p] where
    # s = where(adj, leaky_relu(att), -17) for this core's (head, row-half);
    # sQ holds u8 codes (s = QSC*code + QLO, code 0 = masked), sT f16.
    sQ = nc.dram_tensor("sQ", [U8TILES, 128, N], u8, kind="ExternalInput")
    sT = nc.dram_tensor("sT", [RBLKS - U8TILES, 128, N], f16,
                        kind="ExternalInput")
    xb16 = nc.dram_tensor("xb16", [128, KC * DP1], f16, kind="ExternalInput")
    out = nc.dram_tensor("out", [R, D], f16, kind="ExternalOutput")
    with tile.TileContext(nc) as tc, ExitStack() as ctx:
        _emit(ctx, tc, sQ.ap(), sT.ap(), xb16.ap(), out.ap())
    nc.compile()
    return nc


_PROGRAM = None


def _get_program():
    global _PROGRAM
    if _PROGRAM is None:
        _PROGRAM = _build()
    return _PROGRAM


def _tile_T(a):
    """[rows=n*128, N] -> [n, 128(p), KC*128] with
    out[rb, p, k*128 + r] = a[rb*128 + r, k*128 + p]."""
    nb = a.shape[0] // 128
    rb = a.reshape(nb, 128, KC, 128)             # [rb, r, k, p]
    return np.ascontiguousarray(rb.transpose(0, 3, 2, 1)).reshape(nb, 128, N)


def make_in_maps(x, adj, att_pattern):
    x = np.asarray(x, dtype=np.float32)
    adjm = np.asarray(adj) != 0

    # [H, N, D+1] fp16 with ones column, pre-arranged to the SBUF layout
    # [H, 128, KC*(D+1)] so each head is one contiguous-per-partition DMA.
    xaug = np.empty((H, N, DP1), dtype=np.float16)
    xaug[:, :, :D] = x.astype(np.float16)
    xaug[:, :, D] = np.float16(1.0)
    xb16 = np.ascontiguousarray(
        xaug.reshape(H, KC, 128, DP1).transpose(0, 2, 1, 3).reshape(H, 128, KC * DP1)
    )

    RQ = U8TILES * 128
    in_maps = []
    for c in range(NCORES):
        h, half = c // 2, c % 2
        ap = np.asarray(att_pattern[h], dtype=np.float32)[half * R:(half + 1) * R]
        am = adjm[half * R:(half + 1) * R]
        lk = np.where(ap > 0, ap, np.float32(0.2) * ap)
        codes = np.where(
            am[:RQ],
            np.clip(np.round((lk[:RQ] - QLO) / QSC), 1, 255),
            0).astype(np.uint8)
        s16 = np.where(am[RQ:], lk[RQ:], np.float32(NEG)).astype(np.float16)
        in_maps.append({
            "sQ": _tile_T(codes),
            "sT": _tile_T(s16),
            "xb16": xb16[h],
        })
    return in_maps


def assemble(res):
    full = np.empty((H, N, D), dtype=np.float32)
    for c in range(NCORES):
        h, half = c // 2, c % 2
        full[h, half * R:(half + 1) * R] = res.results[c]["out"]
    return full


def kernel(x, adj, att_pattern, is_val=0, epoch=1, layer_position=0,
           **_unused):
    nc = _get_program()
    in_maps = make_in_maps(x, adj, att_pattern)
    res = run_bass_kernel_spmd(nc, in_maps, core_ids=list(range(NCORES)))
    return assemble(res)


# revision 44
# speedup vs baseline: 1.2042x; 1.2042x over previous
"""Trainium2 Bass kernel for masked-softmax attention (sparse_attention).

Computes, for full inputs
    x           [H=4, N=4096, D=256] f32
    adj         [N, N] int32 (0/1)
    att_pattern [H, N, N] f32
the reference
    score = leaky_relu(att_pattern, 0.2)
    score = where(adj > 0, score, -9e15)
    ratio = softmax(score, axis=-1)
    out   = einsum('hnm,hmd->hnd', ratio, x)

Sharding: head-parallel — core c owns head c//2, row half c%2 (2048 rows of one
head). Each core reads only its own head's x slab (2.1MB instead of the full
8.4MB), and adj is never shipped: the host folds mask+leaky into the score
tensor s = where(adj, leaky_relu(att), -17), which also removes the on-chip
mask multiply and leaky passes entirely.

The kernel is ACT-bound (exp is 1 elem/cycle/lane at 1.2GHz; 8.4M elems/core
~ 59us) with the PE matmul stream just behind it (~58us), so everything else
must hide under the exp stream:
  - the first U8TILES row-blocks ship as uint8 codes (half the f16 bytes);
    the ACT free affine (out = exp(scale*u + bias)) decodes them at zero cost,
    so the DMA ramp feeds the exp stream without stalling it. Quantization
    touches U8TILES/16 of the rows -> l2 err ~8e-3 within the 2e-2 budget.
  - the last block's exp is split [16,12,3,1] contraction-chunks wide so only
    one matmul chunk + a 64KB store trail the final exp; block 0's exp starts
    on a 512-col piece as soon as the first 64KB lands.
  - no PE clock prewarm and no exp pairing: dummy-matmul warmup flips the PE
    HAM gate early but trips the chip power governor (whole-chip ~20%
    downclock); paired 8192-wide ACTIVATEs create >3.4us PE idle bubbles
    that re-throttle the PE clock. Both measured net losses.
  - output stores ride the second HWDGE ring (nc.scalar) so they never
    displace input tiles on the main FIFO.

Per-core pipeline, per 128-row block b (16 blocks):
    DMA   s[b]  [128, 4096] u8/f16
    ACT   e = exp(scale * s[b] + bias)    (~3.7us)  <- bottleneck engine
    PE    psum[128, 257] = sum_k e_chunk.T @ x_chunk  (32 matmuls, ~3.6us)
    DVE   out_rows = psum[:, :256] * (1 / psum[:, 256])
    DMA   store out rows (batched)
fp16 data path, fp32 PSUM accumulation, f16 output (cast f32 on host).
"""

import numpy as np

import concourse.bass as bass
import concourse.mybir as mybir
import concourse.tile as tile
from concourse import bacc
from concourse.bass_utils import run_bass_kernel_spmd

H, N, D = 4, 4096, 256
NCORES = 8
R = N // 2               # rows per core = 2048 (one head, half the rows)
RBLKS = R // 128         # 128-row blocks per core = 16
KC = N // 128            # contraction chunks = 32
DP1 = D + 1              # matmul rhs width (ones column appended)
NEG = -17.0              # masked score: exp(-17) ~ 4e-8 -> 0 in f16

U8TILES = 8              # leading row-blocks shipped as u8 codes
PREWARM = 0              # dummy matmuls before the real stream (PE clock warmup)
                         # (any value that keeps the PE busy early trips the
                         # chip power governor -> whole-chip downclock; keep 0)
QLO, QHI = -6.2, 5.3     # u8 code range; code 0 = masked (exp(QLO) ~ 2e-3)
QSC = (QHI - QLO) / 254.0

f32 = mybir.dt.float32
f16 = mybir.dt.float16
u8 = mybir.dt.uint8
AF = mybir.ActivationFunctionType
OP = mybir.AluOpType


def _emit(ctx, tc: tile.TileContext, sQ: bass.AP, sT: bass.AP, xb16: bass.AP,
          out: bass.AP):
    nc = tc.nc

    qp = ctx.enter_context(tc.tile_pool(name="qp", bufs=1))
    sp = ctx.enter_context(tc.tile_pool(name="sp", bufs=6))
    ep = ctx.enter_context(tc.tile_pool(name="ep", bufs=6))
    xp = ctx.enter_context(tc.tile_pool(name="xp", bufs=1))
    bp = ctx.enter_context(tc.tile_pool(name="bp", bufs=1))
    op = ctx.enter_context(tc.tile_pool(name="op", bufs=2))
    rp = ctx.enter_context(tc.tile_pool(name="rp", bufs=2))
    pp = ctx.enter_context(tc.tile_pool(name="pp", bufs=8, space="PSUM"))

    bt = bp.tile([128, 1], f32, tag="bias")
    nc.vector.memset(bt, QLO)
    # Tiny dummy exp: walrus attaches the ~2.7us exp ACT_TABLE_LOAD to the
    # first ACTIVATE — doing one on a ready tile prefetches the table while
    # the input DMA is still ramping, so the first real exp isn't delayed.
    warm_e = bp.tile([128, 1], f32, tag="warme")
    nc.scalar.activation(warm_e, bt, AF.Exp)

    if PREWARM:
        # PE prewarm: dummy matmuls before any input data arrives, to flip
        # the PE HAM clock-gate early. Left disabled: the chip power governor
        # responds with a whole-chip downclock that costs far more.
        wt = bp.tile([128, 128], f16, tag="warm")
        nc.vector.memset(wt, 0.0)
        wpo = pp.tile([128, DP1], f32, tag="po")
        for _ in range(PREWARM):
            nc.tensor.matmul(wpo[:, :128], lhsT=wt, rhs=wt, start=True,
                             stop=True)

    xs = xp.tile([128, KC, DP1], f16, tag="xs")
    xv = xb16.rearrange("p (k d) -> p k d", k=KC)

    # Input FIFO order: first u8 half starts ACT as early as possible; x
    # chunks are wedged between the u8 tiles just ahead of the PE's needs
    # (the PE runs its first ~3.4us at half clock, so x can trickle).
    sq = qp.tile([128, U8TILES, N], u8, tag="sq")
    HN = N // 2
    QN = 512             # first exp piece: small, so ACT starts ASAP
    # All inputs ride ONE HWDGE ring in need-order. (Measured dead ends: a
    # second concurrent ring round-robins at packet granularity and destroys
    # this ordering, +16us of stalls; 64KB-granularity transfers are
    # descriptor-dominated and halve the early rate.)
    nc.sync.dma_start(sq[:, 0, :QN], sQ[0][:, :QN])
    nc.sync.dma_start(sq[:, 0, QN:HN], sQ[0][:, QN:HN])
    nc.sync.dma_start(sq[:, 0, HN:], sQ[0][:, HN:])
    nc.sync.dma_start(xs[:, :6, :], xv[:, :6, :])
    nc.sync.dma_start(sq[:, 1, :], sQ[1])
    nc.sync.dma_start(xs[:, 6:14, :], xv[:, 6:14, :])
    nc.sync.dma_start(sq[:, 2, :], sQ[2])
    nc.sync.dma_start(xs[:, 14:24, :], xv[:, 14:24, :])
    nc.sync.dma_start(sq[:, 3, :], sQ[3])
    nc.sync.dma_start(xs[:, 24:, :], xv[:, 24:, :])
    for i in range(4, U8TILES):
        nc.sync.dma_start(sq[:, i, :], sQ[i])

    ob = {}

    def norm_store(b, po):
        rec = rp.tile([128, 1], f32, tag="rec")
        nc.vector.reciprocal(rec, po[:, D:DP1])
        if b % 4 == 0:
            ob[0] = op.tile([128, 4, D], f16, tag="o", name=f"ob{b}")
        nc.vector.tensor_scalar_mul(ob[0][:, b % 4, :], po[:, :D], rec)
        if b == RBLKS - 2:
            # ship blocks 12-14 early so only 64KB trails the last block
            nc.scalar.dma_start(
                out[12 * 128:15 * 128].rearrange("(rb p) d -> p rb d", p=128),
                ob[0][:, :3, :])
        elif b == RBLKS - 1:
            # final 64KB store split across both HWDGE rings: the two HBM
            # write-receipt latencies overlap instead of chaining
            nc.scalar.dma_start(
                out[15 * 128:].rearrange("(rb p) d -> p rb d", p=128)[:, :, :D // 2],
                ob[0][:, 3:4, :D // 2])
            nc.sync.dma_start(
                out[15 * 128:].rearrange("(rb p) d -> p rb d", p=128)[:, :, D // 2:],
                ob[0][:, 3:4, D // 2:])
        elif b % 4 == 3:
            g = b // 4
            nc.scalar.dma_start(
                out[g * 512:(g + 1) * 512].rearrange("(rb p) d -> p rb d",
                                                     p=128),
                ob[0])

    def mm(po, e_blk, k0, k1):
        for kk in range(k0, k1):
            nc.tensor.matmul(po, lhsT=e_blk[:, kk * 128:(kk + 1) * 128],
                             rhs=xs[:, kk, :], start=(kk == 0),
                             stop=(kk == KC - 1))

    # u8 zone: singles (supply-limited during the DMA ramp); block 0 halved.
    for b in range(U8TILES):
        st = sq[:, b, :]
        e = ep.tile([128, N], f16, tag="e")
        po = pp.tile([128, DP1], f32, tag="po")
        if b == 0:
            nc.scalar.activation(e[:, :QN], st[:, :QN], AF.Exp, scale=QSC,
                                 bias=bt)
            nc.scalar.activation(e[:, QN:], st[:, QN:], AF.Exp, scale=QSC,
                                 bias=bt)
        else:
            nc.scalar.activation(e, st, AF.Exp, scale=QSC, bias=bt)
        mm(po, e, 0, KC)
        norm_store(b, po)

    # f16 zone: single tiles.
    for b in range(U8TILES, RBLKS - 1):
        st = sp.tile([128, N], f16, tag="s")
        nc.sync.dma_start(st, sT[b - U8TILES])
        e = ep.tile([128, N], f16, tag="e")
        nc.scalar.activation(e, st, AF.Exp)
        po = pp.tile([128, DP1], f32, tag="po")
        mm(po, e, 0, KC)
        norm_store(b, po)

    # last block: uneven exp split so only one matmul chunk trails the exp
    b = RBLKS - 1
    st = sp.tile([128, N], f16, tag="s")
    nc.sync.dma_start(st, sT[b - U8TILES])
    e = ep.tile([128, N], f16, tag="e")
    po = pp.tile([128, DP1], f32, tag="po")
    cuts = [0, 20, 30, KC]
    for ci in range(3):
        k0, k1 = cuts[ci], cuts[ci + 1]
        nc.scalar.activation(e[:, k0 * 128:k1 * 128],
                             st[:, k0 * 128:k1 * 128], AF.Exp)
        mm(po, e, k0, k1)
    norm_store(b, po)


def _build():
    from contextlib import ExitStack

    nc = bacc.Bacc(None, target_bir_lowering=False)
    # s*[rb, p, k*128 + r] = s[rb*128 + r, k*128 + p] where
    # s = where(adj, leaky_relu(att), -17) for this core's (head, row-half);
    # sQ holds u8 codes (s = QSC*code + QLO, code 0 = masked), sT f16.
    sQ = nc.dram_tensor("sQ", [U8TILES, 128, N], u8, kind="ExternalInput")
    sT = nc.dram_tensor("sT", [RBLKS - U8TILES, 128, N], f16,
                        kind="ExternalInput")
    xb16 = nc.dram_tensor("xb16", [128, KC * DP1], f16, kind="ExternalInput")
    out = nc.dram_tensor("out", [R, D], f16, kind="ExternalOutput")
    with tile.TileContext(nc) as tc, ExitStack() as ctx:
        _emit(ctx, tc, sQ.ap(), sT.ap(), xb16.ap(), out.ap())
    nc.compile()
    return nc


_PROGRAM = None


def _get_program():
    global _PROGRAM
    if _PROGRAM is None:
        _PROGRAM = _build()
    return _PROGRAM


def _tile_T(a):
    """[rows=n*128, N] -> [n, 128(p), KC*128] with
    out[rb, p, k*128 + r] = a[rb*128 + r, k*128 + p]."""
    nb = a.shape[0] // 128
    rb = a.reshape(nb, 128, KC, 128)             # [rb, r, k, p]
    return np.ascontiguousarray(rb.transpose(0, 3, 2, 1)).reshape(nb, 128, N)


def make_in_maps(x, adj, att_pattern):
    x = np.asarray(x, dtype=np.float32)
    adjm = np.asarray(adj) != 0

    # [H, N, D+1] fp16 with ones column, pre-arranged to the SBUF layout
    # [H, 128, KC*(D+1)] so each head is one contiguous-per-partition DMA.
    xaug = np.empty((H, N, DP1), dtype=np.float16)
    xaug[:, :, :D] = x.astype(np.float16)
    xaug[:, :, D] = np.float16(1.0)
    xb16 = np.ascontiguousarray(
        xaug.reshape(H, KC, 128, DP1).transpose(0, 2, 1, 3).reshape(H, 128, KC * DP1)
    )

    RQ = U8TILES * 128
    in_maps = []
    for c in range(NCORES):
        h, half = c // 2, c % 2
        ap = np.asarray(att_pattern[h], dtype=np.float32)[half * R:(half + 1) * R]
        am = adjm[half * R:(half + 1) * R]
        lk = np.where(ap > 0, ap, np.float32(0.2) * ap)
        codes = np.where(
            am[:RQ],
            np.clip(np.round((lk[:RQ] - QLO) / QSC), 1, 255),
            0).astype(np.uint8)
        s16 = np.where(am[RQ:], lk[RQ:], np.float32(NEG)).astype(np.float16)
        in_maps.append({
            "sQ": _tile_T(codes),
            "sT": _tile_T(s16),
            "xb16": xb16[h],
        })
    return in_maps


def assemble(res):
    full = np.empty((H, N, D), dtype=np.float32)
    for c in range(NCORES):
        h, half = c // 2, c % 2
        full[h, half * R:(half + 1) * R] = res.results[c]["out"]
    return full


def kernel(x, adj, att_pattern, is_val=0, epoch=1, layer_position=0,
           **_unused):
    nc = _get_program()
    in_maps = make_in_maps(x, adj, att_pattern)
    res = run_bass_kernel_spmd(nc, in_maps, core_ids=list(range(NCORES)))
    return assemble(res)
